# revision 13
# baseline (speedup 1.0000x reference)
"""MoE SwiGLU MLP kernel for 8 Trainium2 NeuronCores.

Problem (hardcoded): x[4,512,1024], E=8 experts, H=2048, shared=1, top-k=2.

Strategy: the host computes the (tiny) gate — softmax + top-2 routing +
combine weights cw[N,E] + balance loss — and dispatches tokens to cores
(expert-parallel with load balancing). Each core runs SwiGLU MLPs over its
token segments in transposed layout (activations [feature, token]) and the
host scatter-adds the per-expert partial outputs.

Two modes:
  - routed (default): cores 0-6 process routed expert c+1's tokens as
    segment A; core 7's segment A and everyone's segment B carry slices of
    the shared expert's 2048 tokens, balancing every core to ~768 real
    token-slots (= total pairs 6144 / 8).
  - dense: every core computes its expert over all 2048 tokens (masked by
    cw); used as a correctness/perf baseline.

Matmuls run as float32r (FP22 mantissa, fp32 accumulate): 4x the fp32 PE
rate at free dims >= 256.
"""

from contextlib import ExitStack

import numpy as np

import concourse.bass as bass
import concourse.mybir as mybir
import concourse.tile as tile
from concourse import bacc

B, T, C = 4, 512, 1024
E, H = 8, 2048
N_SHARED, K = 1, 2
ROUTE_SCALE = 1.0
NTOK = B * T
N_CORES = 8
TARGET = (NTOK * (N_SHARED + K)) // N_CORES  # 768 real slots per core

MODE = "routed"  # or "dense"

F32 = mybir.dt.float32
F32R = mybir.dt.float32r
ALU = mybir.AluOpType
ACT_SILU = mybir.ActivationFunctionType.Silu

LAST_RESULTS = None
LAST_IN_MAPS = None

KC = C // 128   # contraction chunks over C
HT = H // 128   # hidden tiles
CT = C // 128   # output tiles


def _chunks(cap: int):
    """Split the token free-dim into matmul chunks of <=512 (>=256 when
    cap allows, for full f32r rate)."""
    n = -(-cap // 512)
    base = cap // n
    rem = cap - base * n
    sizes = [base + (1 if i < rem else 0) for i in range(n)]
    out, off = [], 0
    for s in sizes:
        out.append((off, s))
        off += s
    return out


def _build_moe(caps) -> bass.Bass:
    """Program with one SwiGLU MLP segment per entry of `caps`.

    Segment i has its own inputs xT{i} [C, cap], weights w1{i} [C,H],
    w3{i} [C,H], w2{i} [H,C], biases b1/b3 [H], b2 [C], per-slot combine
    weights cw{i} [1, cap], and output outT{i} [C, cap].
    """
    nc = bacc.Bacc("TRN2", target_bir_lowering=False, num_devices=N_CORES)

    segs = []
    for i, cap in enumerate(caps):
        s = {
            "cap": cap,
            "xT": nc.declare_dram_parameter(f"xT{i}", [C, cap], F32R, isOutput=False),
            "w1": nc.declare_dram_parameter(f"w1{i}", [C, H], F32R, isOutput=False),
            "w3": nc.declare_dram_parameter(f"w3{i}", [C, H], F32R, isOutput=False),
            "w2": nc.declare_dram_parameter(f"w2{i}", [H, C], F32R, isOutput=False),
            "b1": nc.declare_dram_parameter(f"b1{i}", [H], F32, isOutput=False),
            "b3": nc.declare_dram_parameter(f"b3{i}", [H], F32, isOutput=False),
            "b2": nc.declare_dram_parameter(f"b2{i}", [C], F32, isOutput=False),
            "cw": nc.declare_dram_parameter(f"cw{i}", [1, cap], F32, isOutput=False),
            "outT": nc.declare_dram_parameter(f"outT{i}", [C, cap], F32, isOutput=True),
        }
        segs.append(s)

    with tile.TileContext(nc) as tc, ExitStack() as ctx:
        xp = ctx.enter_context(tc.tile_pool(name="xp", bufs=1))
        hp = ctx.enter_context(tc.tile_pool(name="hp", bufs=1))
        wap = ctx.enter_context(tc.tile_pool(name="wap", bufs=3))
        wgp = ctx.enter_context(tc.tile_pool(name="wgp", bufs=3))
        w2p = ctx.enter_context(tc.tile_pool(name="w2p", bufs=3))
        cwp = ctx.enter_context(tc.tile_pool(name="cwp", bufs=2))
        bp = ctx.enter_context(tc.tile_pool(name="bp", bufs=2))
        actp = ctx.enter_context(tc.tile_pool(name="actp", bufs=4))
        outp = ctx.enter_context(tc.tile_pool(name="outp", bufs=4))
        psA = ctx.enter_context(tc.tile_pool(name="psA", bufs=2, space="PSUM"))
        psG = ctx.enter_context(tc.tile_pool(name="psG", bufs=2, space="PSUM"))
        psO = ctx.enter_context(tc.tile_pool(name="psO", bufs=4, space="PSUM"))

        max_cap = max(caps)

        for s in segs:
            cap = s["cap"]
            chunks = _chunks(cap)

            b1s = bp.tile([128, HT], F32, tag="b1")
            nc.sync.dma_start(out=b1s, in_=s["b1"][:].rearrange("(i p) -> p i", p=128))
            b3s = bp.tile([128, HT], F32, tag="b3")
            nc.sync.dma_start(out=b3s, in_=s["b3"][:].rearrange("(i p) -> p i", p=128))
            b2s = bp.tile([128, CT], F32, tag="b2")
            nc.sync.dma_start(out=b2s, in_=s["b2"][:].rearrange("(j p) -> p j", p=128))

            xTr = s["xT"][:, :].rearrange("(k p) n -> p k n", p=128)
            w1r = s["w1"][:, :].rearrange("(k p) h -> p k h", p=128)
            w3r = s["w3"][:, :].rearrange("(k p) h -> p k h", p=128)
            w2r = s["w2"][:, :].rearrange("(i p) c -> p i c", p=128)

            xts = []
            for k in range(KC):
                xt = xp.tile([128, max_cap], F32R, tag=f"x{k}")
                nc.sync.dma_start(out=xt[:, :cap], in_=xTr[:, k, :])
                xts.append(xt)
            cwbs = []
            for ci, (off, sz) in enumerate(chunks):
                cwb = cwp.tile([128, 512], F32, tag=f"cwb{ci}")
                src = s["cw"][0:1, off:off + sz]
                nc.sync.dma_start(out=cwb[:, :sz], in_=src.to_broadcast([128, sz]))
                cwbs.append(cwb)

            hts = []
            for i in range(HT):
                w1t = wap.tile([128, KC, 128], F32R)
                nc.sync.dma_start(out=w1t, in_=w1r[:, :, i * 128:(i + 1) * 128])
                w3t = wgp.tile([128, KC, 128], F32R)
                nc.sync.dma_start(out=w3t, in_=w3r[:, :, i * 128:(i + 1) * 128])
                ht = hp.tile([128, max_cap], F32R, tag=f"h{i}")
                pas, pgs = [], []
                for (off, sz) in chunks:
                    pas.append(psA.tile([128, sz], F32))
                    pgs.append(psG.tile([128, sz], F32))
                # k outer, chunk inner: consecutive matmuls share lhsT
                for k in range(KC):
                    for ci, (off, sz) in enumerate(chunks):
                        nc.tensor.matmul(
                            pas[ci], lhsT=w1t[:, k, :], rhs=xts[k][:, off:off + sz],
                            start=(k == 0), stop=(k == KC - 1))
                for k in range(KC):
                    for ci, (off, sz) in enumerate(chunks):
                        nc.tensor.matmul(
                            pgs[ci], lhsT=w3t[:, k, :], rhs=xts[k][:, off:off + sz],
                            start=(k == 0), stop=(k == KC - 1))
                for ci, (off, sz) in enumerate(chunks):
                    sil = actp.tile([128, 512], F32)
                    nc.scalar.activation(sil[:, :sz], pgs[ci], ACT_SILU,
                                         bias=b3s[:, i:i + 1])
                    # h = (a + b1) * silu(g + b3)
                    nc.vector.scalar_tensor_tensor(
                        out=ht[:, off:off + sz], in0=pas[ci],
                        scalar=b1s[:, i:i + 1], in1=sil[:, :sz],
                        op0=ALU.add, op1=ALU.mult)
                hts.append(ht)

            for j in range(CT):
                w2t = w2p.tile([128, HT, 128], F32R)
                nc.sync.dma_start(out=w2t, in_=w2r[:, :, j * 128:(j + 1) * 128])
                psos = [psO.tile([128, sz], F32) for (off, sz) in chunks]
                for i in range(HT):
                    for ci, (off, sz) in enumerate(chunks):
                        nc.tensor.matmul(
                            psos[ci], lhsT=w2t[:, i, :], rhs=hts[i][:, off:off + sz],
                            start=(i == 0), stop=(i == HT - 1))
                for ci, (off, sz) in enumerate(chunks):
                    ot = outp.tile([128, 512], F32)
                    # out = (o + b2) * cw
                    nc.vector.scalar_tensor_tensor(
                        out=ot[:, :sz], in0=psos[ci], scalar=b2s[:, j:j + 1],
                        in1=cwbs[ci][:, :sz], op0=ALU.add, op1=ALU.mult)
                    nc.sync.dma_start(
                        out=s["outT"][j * 128:(j + 1) * 128, off:off + sz],
                        in_=ot[:, :sz])
    nc.compile()
    return nc


_PROGRAMS = {}
_RUNNERS = {}


def _get_runner(caps):
    caps = tuple(caps)
    if caps not in _RUNNERS:
        if caps not in _PROGRAMS:
            _PROGRAMS[caps] = _build_moe(list(caps))
        _RUNNERS[caps] = _CachedSpmdRunner(_PROGRAMS[caps], N_CORES)
    return _RUNNERS[caps]


class _CachedSpmdRunner:
    """Compile the bass program through PJRT once; reuse across calls."""

    def __init__(self, nc, n_cores: int):
        import jax
        from jax.sharding import Mesh, PartitionSpec
        from jax.experimental.shard_map import shard_map
        from concourse import bass2jax

        bass2jax.install_neuronx_cc_hook()
        self.nc = nc
        self.n_cores = n_cores

        partition_name = (
            nc.partition_id_tensor.name if nc.partition_id_tensor else None
        )
        in_names, out_names, out_avals, zero_outs = [], [], [], []
        for alloc in nc.m.functions[0].allocations:
            if not isinstance(alloc, mybir.MemoryLocationSet):
                continue
            name = alloc.memorylocations[0].name
            if alloc.kind == "ExternalInput":
                if name != partition_name:
                    in_names.append(name)
            elif alloc.kind == "ExternalOutput":
                shape = tuple(alloc.tensor_shape)
                dtype = mybir.dt.np(alloc.dtype)
                out_names.append(name)
                out_avals.append(jax.core.ShapedArray(shape, dtype))
                zero_outs.append(np.zeros(shape, dtype))
        self.n_params = len(in_names)
        n_outs = len(out_avals)
        self.param_names = list(in_names)
        self.out_names = out_names
        self.out_avals = out_avals
        self.zero_outs = zero_outs
        in_names = in_names + out_names
        if partition_name is not None:
            in_names.append(partition_name)

        def _body(*args):
            from concourse.bass2jax import _bass_exec_p, partition_id_tensor

            operands = list(args)
            if partition_name is not None:
                operands.append(partition_id_tensor())
            outs = _bass_exec_p.bind(
                *operands,
                out_avals=tuple(out_avals),
                in_names=tuple(in_names),
                out_names=tuple(out_names),
                lowering_input_output_aliases=(),
                sim_require_finite=True,
                sim_require_nnan=True,
                nc=nc,
            )
            return tuple(outs)

        devices = jax.devices()[:n_cores]
        assert len(devices) == n_cores
        mesh = Mesh(np.asarray(devices), ("core",))
        in_specs = (PartitionSpec("core"),) * (self.n_params + n_outs)
        out_specs = (PartitionSpec("core"),) * n_outs
        self.mesh = mesh
        # No donation: the kernel writes every output element, so the
        # pre-zeroed output buffers can be reused across benchmark calls.
        self.sharded = jax.jit(
            shard_map(_body, mesh=mesh, in_specs=in_specs, out_specs=out_specs,
                      check_rep=False),
            keep_unused=True,
        )

    def concat_inputs(self, in_maps):
        concat_in = [
            np.concatenate([np.asarray(in_maps[c][n]) for c in range(self.n_cores)],
                           axis=0)
            for n in self.param_names
        ]
        concat_zeros = [
            np.zeros((self.n_cores * z.shape[0], *z.shape[1:]), z.dtype)
            for z in self.zero_outs
        ]
        return concat_in, concat_zeros

    def run_raw(self, concat_in, concat_zeros):
        return self.sharded(*concat_in, *concat_zeros)

    def run(self, in_maps):
        concat_in, concat_zeros = self.concat_inputs(in_maps)
        out_arrs = self.run_raw(concat_in, concat_zeros)
        return [
            {
                name: np.asarray(out_arrs[i]).reshape(
                    self.n_cores, *self.out_avals[i].shape)[c]
                for i, name in enumerate(self.out_names)
            }
            for c in range(self.n_cores)
        ]


def _host_gate(xf, gate_w, gate_b):
    """Replicates the reference gating in numpy.

    Returns cw [N, E] combine weights and the balance loss."""
    logits = xf.astype(np.float32) @ gate_w.astype(np.float32) + gate_b
    m = logits.max(axis=-1, keepdims=True)
    ex = np.exp(logits - m)
    scores = ex / ex.sum(axis=-1, keepdims=True)

    routed = scores[:, N_SHARED:]
    # top-k with ties broken toward the lowest index (lax.top_k semantics)
    order = np.argsort(-routed, axis=1, kind="stable")[:, :K]
    topv = np.take_along_axis(routed, order, axis=1)

    n = xf.shape[0]
    rows = np.arange(n)[:, None]
    cw = np.zeros((n, E), np.float32)
    cw[:, :N_SHARED] = scores[:, :N_SHARED] * ROUTE_SCALE
    cw[rows, order + N_SHARED] = topv * ROUTE_SCALE

    all_idx = np.concatenate(
        [np.broadcast_to(np.arange(N_SHARED), (n, N_SHARED)), order + N_SHARED],
        axis=1,
    ).reshape(-1)
    all_w = np.concatenate(
        [scores[:, :N_SHARED] * ROUTE_SCALE, topv * ROUTE_SCALE], axis=1
    ).reshape(-1)
    usage = np.zeros(E, np.float64)
    np.add.at(usage, all_idx, 1.0)
    score_sum = np.zeros(E, np.float64)
    np.add.at(score_sum, all_idx, all_w.astype(np.float64))
    kp = float(N_SHARED + K)
    bal_loss = np.float32(np.sum((E / (kp * n)) * usage * (score_sum / n)))
    return cw, bal_loss


def _roundup(v, m):
    return -(-v // m) * m


def _plan_routed(cw):
    """Assign token slots to (core, segment) balancing real work.

    Returns (cap_a, cap_b, seg_tokens) where seg_tokens[seg][core] =
    (expert_id, np.ndarray of token indices)."""
    sel = cw > 0.0
    counts = sel[:, 1:].sum(axis=0)  # routed experts 1..7
    n_e = [int(counts[e - 1]) for e in range(1, E)]

    cap_a = max(256, _roundup(max(n_e), 64))
    cap_a = min(cap_a, NTOK)

    # core 7's segment A: first cap_a shared tokens
    shared = np.arange(NTOK)
    rem = NTOK - cap_a
    load = n_e + [cap_a]  # per-core segment-A real counts
    want = np.maximum(0, TARGET - np.asarray(load, np.int64))
    if want.sum() <= 0:
        s_counts = np.zeros(8, np.int64)
        s_counts[:] = rem // 8
        s_counts[: rem - int(s_counts.sum())] += 1
    else:
        frac = want * rem / want.sum()
        s_counts = np.floor(frac).astype(np.int64)
        shortfall = rem - int(s_counts.sum())
        order = np.argsort(-(frac - s_counts))
        for i in range(shortfall):
            s_counts[order[i % 8]] += 1
    assert int(s_counts.sum()) == rem

    cap_b = max(256, _roundup(int(s_counts.max()), 64))

    seg_a, seg_b = [], []
    off = cap_a
    for c in range(N_CORES):
        if c < 7:
            toks = np.nonzero(sel[:, c + 1])[0]
            seg_a.append((c + 1, toks))
        else:
            seg_a.append((0, shared[:cap_a]))
        sc = int(s_counts[c])
        seg_b.append((0, shared[off:off + sc]))
        off += sc
    assert off == NTOK
    return cap_a, cap_b, [seg_a, seg_b]


def _seg_inputs(name_idx, expert, toks, cap, xT, cw, w1, b1, w2, b2, w3, b3):
    n = len(toks)
    xa = np.zeros((C, cap), np.float32)
    xa[:, :n] = xT[:, toks]
    cwa = np.zeros((1, cap), np.float32)
    cwa[0, :n] = cw[toks, expert]
    return {
        f"xT{name_idx}": xa,
        f"w1{name_idx}": w1[expert],
        f"w3{name_idx}": w3[expert],
        f"w2{name_idx}": w2[expert],
        f"b1{name_idx}": b1[expert],
        f"b3{name_idx}": b3[expert],
        f"b2{name_idx}": b2[expert],
        f"cw{name_idx}": cwa,
    }


def kernel(x, gate_w, gate_b, w1, b1, w2, b2, w3, b3):
    global LAST_RESULTS, LAST_IN_MAPS
    x = np.asarray(x, np.float32)
    gate_w = np.asarray(gate_w, np.float32)
    gate_b = np.asarray(gate_b, np.float32)
    w1 = np.ascontiguousarray(np.asarray(w1, np.float32))
    b1 = np.ascontiguousarray(np.asarray(b1, np.float32))
    w2 = np.ascontiguousarray(np.asarray(w2, np.float32))
    b2 = np.ascontiguousarray(np.asarray(b2, np.float32))
    w3 = np.ascontiguousarray(np.asarray(w3, np.float32))
    b3 = np.ascontiguousarray(np.asarray(b3, np.float32))

    xf = x.reshape(NTOK, C)
    cw, bal_loss = _host_gate(xf, gate_w, gate_b)
    xT = np.ascontiguousarray(xf.T)  # [C, NTOK]

    if MODE == "dense":
        half = NTOK // 2
        caps = (half, half)
        in_maps = []
        for c in range(N_CORES):
            m = {}
            for i, sl in enumerate((slice(0, half), slice(half, NTOK))):
                toks = np.arange(NTOK)[sl]
                m.update(_seg_inputs(i, c, toks, half, xT, cw,
                                     w1, b1, w2, b2, w3, b3))
            in_maps.append(m)
        seg_tokens = [
            [(c, np.arange(0, half)) for c in range(N_CORES)],
            [(c, np.arange(half, NTOK)) for c in range(N_CORES)],
        ]
    else:
        cap_a, cap_b, seg_tokens = _plan_routed(cw)
        caps = (cap_a, cap_b)
        in_maps = []
        for c in range(N_CORES):
            m = {}
            for i, cap in enumerate(caps):
                expert, toks = seg_tokens[i][c]
                m.update(_seg_inputs(i, expert, toks, cap, xT, cw,
                                     w1, b1, w2, b2, w3, b3))
            in_maps.append(m)

    LAST_IN_MAPS = in_maps
    runner = _get_runner(caps)
    results = runner.run(in_maps)
    LAST_RESULTS = results

    acc = np.zeros((NTOK, C), np.float64)
    for i in range(len(caps)):
        for c in range(N_CORES):
            _, toks = seg_tokens[i][c]
            part = results[c][f"outT{i}"][:, :len(toks)]
            acc[toks] += part.T  # toks unique within one (segment, core)
    out = acc.astype(np.float32).reshape(B, T, C)
    return out, bal_loss
